# revision 4
# baseline (speedup 1.0000x reference)
"""Causal self-attention (B=4, T=2048, C=1024, H=16, D=64) on 8 trn2 cores.

Sharding: core c -> (batch b = c//2, head-group g = c%2); a head group is
8 heads = 512 feature columns of each of Q/K/V.  Per core, one fully
software-pipelined program:

  - QKV projection blocks produce Q^T/K^T [64,2048] fp16 per head and
    V [2048,64] fp16 (+ a ones column that makes the AV matmul emit the
    softmax denominator for free).
  - Scores stay transposed (S^T[k,q]) so exp(S^T) feeds the AV matmul as
    the moving operand with no transposes anywhere.
  - Off-diagonal AV matmuls run in fp8e4 DoubleRow mode: the exp writes
    exp(s-1) straight to an fp8 pair tile (bias keeps values under the
    e4m3 max of 240 and cancels in the softmax), and one matmul contracts
    TWO key blocks at once against an interleaved fp8 copy of V.
    Diagonal blocks and all of qi=0 stay fp16.
  - The whole attention sweep is ONE flat software-pipelined stream over
    (qi, pr, kc) with the score matmul running one iteration ahead
    (across pr and qi boundaries), so the exp stream never waits for a
    pipeline restart.
  - Projection / out-projection matmuls are drip-fed between attention
    iterations at a per-qi rate; a dependency-driven need() forcing
    advances the drip exactly when an attention instruction requires a
    produced tile, so there are no bulk drains.
  - Engine balance: exp owns ACT; q/k bias-apply and tb0's V copies run
    on the otherwise-idle Scalar engine early on; the fp8 V copy runs on
    GpSimd (from SBUF); masks apply to both heads in one DVE op.

Host pre-arranges inputs partition-major (fp16; W_q/W_k dc-major so the
first dc chunk can DMA first) and sums the two per-batch partials,
folding b_out + b_v @ W_out (exact: softmax rows sum to 1).
"""

from collections import deque
from contextlib import ExitStack

import numpy as np

import concourse.bass as bass
import concourse.mybir as mybir
import concourse.tile as tile
from concourse import bacc
from concourse import bass_utils

F32 = mybir.dt.float32
F16 = mybir.dt.float16
F8 = mybir.dt.float8e4
EXPB = -1.0      # exp(s + EXPB): keeps fp8 et below e4m3 max (240); softmax-invariant
NF8PAIR = 6      # kc-block pairs 0..5 (blocks 0-11) get an fp8 copy of V

B, T, C = 4, 2048, 1024
H, D = 16, 64
G = 2            # head groups (cores per batch)
HPG = 8          # heads per group
CPH = HPG * D    # feature columns per group = 512
N = 512          # matmul moving free dim
NCORES = 8
SCALE = 1.0 / np.sqrt(D)
Exp = mybir.ActivationFunctionType.Exp
Ident = mybir.ActivationFunctionType.Identity

_CACHE = {}


def _build_program():
    if "nc" in _CACHE:
        return _CACHE["nc"]

    nc = bacc.Bacc("TRN2", target_bir_lowering=False, debug=False, num_devices=NCORES)

    # all inputs pre-arranged host-side: partition-major, fp16
    xTr = nc.dram_tensor("xTr", [128, 4, 8, N], F16, kind="ExternalInput").ap()
    wqr = nc.dram_tensor("wqr", [128, 4, 8, 128], F16, kind="ExternalInput").ap()
    wkr = nc.dram_tensor("wkr", [128, 4, 8, 128], F16, kind="ExternalInput").ap()
    wvr = nc.dram_tensor("wvr", [128, 8, CPH], F16, kind="ExternalInput").ap()
    bqr = nc.dram_tensor("bqr", [128, 4], F32, kind="ExternalInput").ap()
    bkr = nc.dram_tensor("bkr", [128, 4], F32, kind="ExternalInput").ap()  # ×SCALE
    wor = nc.dram_tensor("wor", [128, 4, C], F16, kind="ExternalInput").ap()
    masks = nc.dram_tensor("masks", [128, 4, N], F16, kind="ExternalInput").ap()
    yp = nc.dram_tensor("yp", [T, C], F16, kind="ExternalOutput").ap()

    with tile.TileContext(nc) as tc, ExitStack() as ctx:
        wpool = ctx.enter_context(tc.tile_pool(name="wpool", bufs=1))
        big = ctx.enter_context(tc.tile_pool(name="big", bufs=1))
        epool = ctx.enter_context(tc.tile_pool(name="et", bufs=6))
        e8pool = ctx.enter_context(tc.tile_pool(name="et8", bufs=3))
        mpool = ctx.enter_context(tc.tile_pool(name="mpool", bufs=4))
        blkps = ctx.enter_context(tc.tile_pool(name="blkps", bufs=2, space="PSUM"))
        sps = ctx.enter_context(tc.tile_pool(name="sps", bufs=2, space="PSUM"))
        avps = ctx.enter_context(tc.tile_pool(name="avps", bufs=1, space="PSUM"))

        XT = big.tile([128, 4, 8, N], F16)   # x^T resident (tb-major, c-chunks)
        QT = big.tile([128, 4, T], F16)   # Q^T (+bias)
        KT = big.tile([128, 4, T], F16)   # SCALE * (K^T + bias)
        VA = big.tile([128, 16, HPG, D + 1], F16)   # V rows + ones column
        # fp8 copy of V for off-diagonal AV: kc-block pairs interleaved for
        # DoubleRow ([partition, pair, j, head, d]; d padded to 66 so the
        # j-stride (8*66=528) is 16-byte aligned)
        VA8 = big.tile([128, NF8PAIR, 2, HPG, 66], F8)
        ON = big.tile([128, 4, T], F16)   # normalized O^T (c_in x tokens)

        WQ = wpool.tile([128, 4, 8, 128], F16)   # dc-major
        WK = wpool.tile([128, 4, 8, 128], F16)
        WV = wpool.tile([128, 8, CPH], F16)
        BQ = wpool.tile([128, 4], F32)
        BKs = wpool.tile([128, 4], F32)
        MS = wpool.tile([128, 4, N], F16)
        BEX = wpool.tile([128, 1], F32)   # exp bias (EXPB) as per-partition AP
        SCR = wpool.tile([128, N], F16)   # scratch for PE clock warm-up
        WO = wpool.tile([128, 4, C], F16)

        # PE clock warm-up: the HAM gate starts at 1.2GHz and needs ~3.4us
        # of sustained activity to release to 2.4GHz.  PE is idle until the
        # first weights land, so dummy matmuls on a memset scratch tile warm
        # it for free (outputs are never read).
        nc.any.memset(SCR[:], 0.0)
        wps = blkps.tile([128, N], F32, name="blk")
        for _ in range(9):
            nc.tensor.matmul(wps[:], SCR[:, 0:128], SCR[:], start=True, stop=True)

        # input DMAs ordered by first use: the first score pair only needs
        # WQ/WK dc0 + x tokens of tb0, so those stream first (x in c-chunk
        # pairs so q0's matmuls start before the whole tb lands)
        nc.sync.dma_start(BQ[:], bqr)
        nc.sync.dma_start(BKs[:], bkr)
        nc.sync.dma_start(WQ[:, 0], wqr[:, 0])
        for cc2 in range(4):
            sl = slice(2 * cc2, 2 * cc2 + 2)
            nc.sync.dma_start(XT[:, 0, sl, :], xTr[:, 0, sl, :])
        nc.sync.dma_start(WK[:, 0], wkr[:, 0])
        nc.sync.dma_start(MS[:], masks)
        for dc in range(1, 4):
            nc.sync.dma_start(WQ[:, dc], wqr[:, dc])
            nc.sync.dma_start(WK[:, dc], wkr[:, dc])
        nc.sync.dma_start(WV[:], wvr)
        nc.any.memset(VA[:, :, :, D : D + 1], 1.0)
        nc.any.memset(VA8[:, :, :, :, D : D + 1], 1.0)
        nc.any.memset(BEX[:], EXPB)
        for tb in range(1, 4):
            nc.sync.dma_start(XT[:, tb, :, :], xTr[:, tb, :, :])
        nc.sync.dma_start(WO[:], wor)

        def qkv_block_gen(tb, which, dc):
            """Generator: one projection block, yielding every 2 matmuls."""
            ps = blkps.tile([128, N], F32, name="blk")
            if which == "v":
                for cc in range(8):
                    nc.tensor.matmul(
                        ps[:],
                        XT[:, tb, cc, dc * 128 : (dc + 1) * 128],
                        WV[:, cc],
                        start=(cc == 0),
                        stop=(cc == 7),
                    )
                    if cc % 2 == 1:
                        yield
                bi = tb * 4 + dc
                src = ps[:].rearrange("p (h d) -> p h d", h=HPG)
                if tb == 0:
                    nc.scalar.copy(VA[:, bi, :, 0:D], src)
                else:
                    nc.vector.tensor_copy(VA[:, bi, :, 0:D], src)
                if bi < 2 * NF8PAIR:
                    # fp8 interleaved copy runs on GpSimd from the fp16
                    # SBUF copy (GpSimd has no PSUM port), off everyone's
                    # critical path
                    nc.gpsimd.tensor_copy(
                        VA8[:, bi // 2, bi % 2, :, 0:D], VA[:, bi, :, 0:D]
                    )
            else:
                WT, dst, scl, bias = (
                    (WQ, QT, 1.0, BQ) if which == "q" else (WK, KT, SCALE, BKs)
                )
                for cc in range(8):
                    nc.tensor.matmul(
                        ps[:],
                        WT[:, dc, cc, :],
                        XT[:, tb, cc, :],
                        start=(cc == 0),
                        stop=(cc == 7),
                    )
                    if cc % 2 == 1:
                        yield
                out_ap = dst[:, dc, tb * N : (tb + 1) * N]
                if tb <= 1:
                    # ACT is idle during the ramp / qi0: bias-apply there
                    nc.scalar.activation(
                        out_ap, ps[:], Ident, bias=bias[:, dc, None], scale=scl
                    )
                else:
                    nc.vector.scalar_tensor_tensor(
                        out=out_ap,
                        in0=ps[:],
                        scalar=scl,
                        in1=bias[:, dc, None].to_broadcast((128, N)),
                        op0=mybir.AluOpType.mult,
                        op1=mybir.AluOpType.add,
                    )

        def y_block_gen(ic, ob, alt=False):
            """Generator: one out-projection block, yielding every 2 matmuls.

            Tail blocks (alt=True) borrow the first bank of the (by then
            idle) score psum tiles so four blocks can rotate in flight."""
            if alt:
                ypt = sps.tile([128, 2 * N], F32, name="sp")[:, 0:N]
            else:
                ypt = blkps.tile([128, N], F32, name="blk")
            for cc4 in range(4):
                nc.tensor.matmul(
                    ypt[:],
                    ON[:, cc4, ic * 128 : (ic + 1) * 128],
                    WO[:, cc4, ob * N : (ob + 1) * N],
                    start=(cc4 == 0),
                    stop=(cc4 == 3),
                )
                if cc4 % 2 == 1:
                    yield
            ysb = mpool.tile([128, N], F16, name="ysb")
            nc.vector.tensor_copy(ysb[:], ypt[:])
            nc.sync.dma_start(
                yp[ic * 128 : (ic + 1) * 128, ob * N : (ob + 1) * N], ysb[:]
            )

        def warm_gen(n):
            # keeps the HAM clock gate warm across DMA-paced prefix gaps
            wp = blkps.tile([128, N], F32, name="blk")
            for _ in range(n):
                nc.tensor.matmul(
                    wp[:], SCR[:, 0:128], SCR[:], start=True, stop=True
                )
                yield

        # ---- drip machinery: FIFO generator queue with labels + need()
        drip = deque()           # (label, generator)
        done_labels = set()

        def drip_advance(n):
            for _ in range(n):
                while drip:
                    lbl, g = drip[0]
                    try:
                        next(g)
                        break
                    except StopIteration:
                        if lbl is not None:
                            done_labels.add(lbl)
                        drip.popleft()
                else:
                    return

        def need(lbl):
            while lbl not in done_labels:
                if not drip:
                    raise RuntimeError(f"need({lbl}) but drip empty")
                drip_advance(1)

        def add(which, tb, dc, warm=0):
            drip.append(((which, tb, dc), qkv_block_gen(tb, which, dc)))
            if warm:
                drip.append((None, warm_gen(warm)))

        # projection generators in due-date order; within tb t the first
        # score pair needs q-dc0 (at qi=t start) then k-dc0 / v-dc* (at
        # pr0's kc=4t), then q/k per pr
        add("q", 0, 0, warm=3)
        add("k", 0, 0, warm=3)
        for dc in range(4):
            add("v", 0, dc, warm=(3 if dc < 2 else 0))
        for dc in range(1, 4):
            add("q", 0, dc)
            add("k", 0, dc)
        for t in range(1, 4):
            add("q", t, 0)
            add("k", t, 0)
            for dc in range(4):
                add("v", t, dc)
            for dc in range(1, 4):
                add("q", t, dc)
                add("k", t, dc)

        # out-projection blocks and the explicit tail (last 8 blocks)
        tail_blocks = [(12 + i4, ob) for i4 in range(4) for ob in range(2)]
        tail_pre = {}

        def tail_prefix_gen(bi, ic, ob):
            # pre-issued during qi=3's last iterations: only needs ON
            # pr0-2, which are ready well before pr3 finishes
            ypt = blkps.tile([128, N], F32, name="blk")
            for cc4 in range(3):
                nc.tensor.matmul(
                    ypt[:],
                    ON[:, cc4, ic * 128 : (ic + 1) * 128],
                    WO[:, cc4, ob * N : (ob + 1) * N],
                    start=(cc4 == 0),
                    stop=False,
                )
                yield
            tail_pre[bi] = ypt[:]

        # ---- flat attention stream -------------------------------------
        iters = [
            (qi, pr, kc)
            for qi in range(4)
            for pr in range(4)
            for kc in range(4 * qi + 4)
        ]
        NIT = len(iters)

        def emit_s(qi, pr, kc):
            # both heads' score tiles in one 2-bank psum tile so the exp
            # runs 1024 wide; the two matmuls run concurrently (row groups
            # 0-1 / 2-3).  Diagonal chunks only compute the causally-
            # reachable column range [vq:].
            need(("q", qi, pr))
            need(("k", kc // 4, pr))
            vq = max(0, (kc - 4 * qi) * 128)
            sp = sps.tile([128, 2 * N], F32, name="sp")
            for hi in range(2):
                off = 64 * hi
                nc.tensor.matmul(
                    sp[:, hi * N + vq : (hi + 1) * N],
                    KT[off : off + 64, pr, kc * 128 : (kc + 1) * 128],
                    QT[off : off + 64, pr, qi * N + vq : (qi + 1) * N],
                    start=True,
                    stop=True,
                )
            return sp

        # drip rate per attention iteration (num/den) by qi, tuned so each
        # token block's projections finish just before they are needed and
        # the out-projection backlog lands in qi2/qi3 ACT slack
        RATE = {0: (4, 1), 1: (5, 4), 2: (5, 4), 3: (3, 2)}

        sp_tiles = {}
        avs = None
        bacc_ctr = 0
        for idx, (qi, pr, kc) in enumerate(iters):
            nkc = 4 * qi + 4
            if kc == 0:
                avs = [avps.tile([D + 1, N], F32, name=f"av{hi}") for hi in range(2)]
                if qi == 3 and pr == 0:
                    # out-projection backlog for qi2 (its ON rows are done)
                    for i4 in range(4):
                        for ob in range(2):
                            drip.append((None, y_block_gen(8 + i4, ob)))
                if qi == 3 and pr == 3:
                    # tail prefixes read ON pr0-2 of qi3: all normalizes
                    # they touch are emitted by now
                    for bi in (0, 2):
                        drip.append((None, tail_prefix_gen(bi, *tail_blocks[bi])))
            if idx not in sp_tiles:
                sp_tiles[idx] = emit_s(qi, pr, kc)
            if idx + 1 < NIT:
                nqi, npr, nkcc = iters[idx + 1]
                sp_tiles[idx + 1] = emit_s(nqi, npr, nkcc)
            sp_cur = sp_tiles.pop(idx)

            vq = max(0, (kc - 4 * qi) * 128)
            nfp8 = 4 * qi
            if kc < nfp8:
                # fp8 path: exp(s-1) into the pair tile; the odd member
                # fires one DoubleRow AV per head contracting both blocks
                if kc % 2 == 0:
                    et2 = e8pool.tile([128, 2, 2, N], F8, name="et2")
                nc.scalar.activation(
                    et2[:, kc % 2, :, :], sp_cur[:], Exp, bias=BEX[:]
                )
                if kc % 2 == 1:
                    need(("v", kc // 4, kc % 4))
                    need(("v", (kc - 1) // 4, (kc - 1) % 4))
                    for hi in range(2):
                        nc.tensor.matmul(
                            avs[hi][:],
                            VA8[:, kc // 2, :, 2 * pr + hi, 0 : D + 1],
                            et2[:, :, hi, :],
                            start=(kc == 1),
                            stop=False,
                            perf_mode=mybir.MatmulPerfMode.DoubleRow,
                        )
            else:
                et = epool.tile([128, 2 * N], F16, name="et")
                if vq == 0:
                    nc.scalar.activation(et[:], sp_cur[:], Exp, bias=BEX[:])
                else:
                    # both heads' [vq:] ranges in one strided-AP
                    # instruction (saves the per-instr ACT overhead)
                    nc.scalar.activation(
                        et[:].rearrange("p (h q) -> p h q", h=2)[:, :, vq:],
                        sp_cur[:].rearrange("p (h q) -> p h q", h=2)[:, :, vq:],
                        Exp,
                        bias=BEX[:],
                    )
                if kc >= 4 * qi:
                    # mask only the 128-wide diagonal sub-block for both
                    # heads in one strided op; the rest of [vq:] is fully
                    # below the diagonal
                    nc.vector.tensor_tensor(
                        et[:].rearrange("p (h q) -> p h q", h=2)[
                            :, :, vq : vq + 128
                        ],
                        et[:].rearrange("p (h q) -> p h q", h=2)[
                            :, :, vq : vq + 128
                        ],
                        MS[:, kc - 4 * qi, None, vq : vq + 128].to_broadcast(
                            (128, 2, 128)
                        ),
                        mybir.AluOpType.mult,
                    )
                need(("v", kc // 4, kc % 4))
                for hi in range(2):
                    nc.tensor.matmul(
                        avs[hi][:, vq:],
                        VA[:, kc, 2 * pr + hi, :],
                        et[:, hi * N + vq : (hi + 1) * N],
                        start=(kc == 0),
                        stop=(kc == nkc - 1),
                    )
            bnum, bden = RATE[qi]
            bacc_ctr += bnum
            drip_advance(bacc_ctr // bden)
            bacc_ctr %= bden

            if kc == nkc - 1:
                last = qi == 3 and pr == 3
                if last:
                    # tail: read PSUM directly (no later user of the banks)
                    # and interleave the two heads' chains to cut latency
                    dns, rbs, rcs = [], [], []
                    for hi in range(2):
                        dn = mpool.tile([1, N], F32, name="dn")
                        nc.vector.tensor_copy(dn[:], avs[hi][D : D + 1, :])
                        dns.append(dn)
                    for hi in range(2):
                        rb = mpool.tile([64, N], F32, name="rb")
                        nc.gpsimd.partition_broadcast(rb[:], dns[hi][:])
                        rbs.append(rb)
                    for hi in range(2):
                        rc = mpool.tile([64, N], F32, name="rc")
                        nc.vector.reciprocal_approx_fast(rc[:], rbs[hi][:])
                        rcs.append(rc)
                    for hi in range(2):
                        off = 64 * hi
                        seg = ON[off : off + 64, pr, qi * N : (qi + 1) * N]
                        nc.vector.tensor_tensor(
                            seg, avs[hi][0:64, :], rcs[hi][:], mybir.AluOpType.mult
                        )
                else:
                    for hi in range(2):
                        off = 64 * hi
                        # one copy releases the accumulator bank; the rest
                        # of the normalize chain runs off SBUF, off the
                        # critical path
                        oc = mpool.tile([D + 1, N], F32, name="oc")
                        nc.vector.tensor_copy(oc[:], avs[hi][:])
                        dn = mpool.tile([1, N], F32, name="dn")
                        nc.vector.tensor_copy(dn[:], oc[D : D + 1, :])
                        rb = mpool.tile([64, N], F32, name="rb")
                        nc.gpsimd.partition_broadcast(rb[:], dn[:])
                        rc = mpool.tile([64, N], F32, name="rc")
                        nc.vector.reciprocal_approx_fast(rc[:], rb[:])
                        seg = ON[off : off + 64, pr, qi * N : (qi + 1) * N]
                        nc.vector.tensor_tensor(
                            seg, oc[0:64, :], rc[:], mybir.AluOpType.mult
                        )
                # out-projection blocks become available at each qi end;
                # qi0/qi1 blocks drip during the next qi's ACT slack
                if pr == 3 and qi < 2:
                    for i4 in range(4):
                        for ob in range(2):
                            drip.append((None, y_block_gen(4 * qi + i4, ob)))
        while drip:
            drip_advance(1)

        # tail: the last query block's out-projections.  Four blocks' first
        # three contraction chunks (they only need ON pr0-2) issue
        # back-to-back across all four free psum slots, so PE has work
        # queued while the last normalize chain produces ON pr3; their
        # final chunks then land in quick succession.  PSUM->SBUF copies
        # run on the by-now idle Scalar engine.
        def tail_psum(bi):
            if bi % 2 == 1:
                return sps.tile([128, 2 * N], F32, name="sp")[:, 0:N]
            return blkps.tile([128, N], F32, name="blk")

        def tail_finish(ypt, ic, ob):
            nc.tensor.matmul(
                ypt,
                ON[:, 3, ic * 128 : (ic + 1) * 128],
                WO[:, 3, ob * N : (ob + 1) * N],
                start=False,
                stop=True,
            )
            ysb = mpool.tile([128, N], F16, name="ysb")
            nc.scalar.copy(ysb[:], ypt)
            nc.sync.dma_start(
                yp[ic * 128 : (ic + 1) * 128, ob * N : (ob + 1) * N], ysb[:]
            )

        pending = []
        for bi, (ic, ob) in enumerate(tail_blocks):
            if bi in tail_pre:
                pending.append((tail_pre[bi], ic, ob))
            else:
                ypt = tail_psum(bi)
                for cc4 in range(3):
                    nc.tensor.matmul(
                        ypt,
                        ON[:, cc4, ic * 128 : (ic + 1) * 128],
                        WO[:, cc4, ob * N : (ob + 1) * N],
                        start=(cc4 == 0),
                        stop=False,
                    )
                pending.append((ypt, ic, ob))
            if bi >= 3:
                # keep at most 4 prefixes in flight (4 psum slots)
                tail_finish(*pending.pop(0))
        for args in pending:
            tail_finish(*args)

    nc.compile()
    _CACHE["nc"] = nc
    return nc


def _make_masks():
    kp = np.arange(128)[:, None]
    qf = np.arange(N)[None, :]
    m = np.stack([(qf >= kp + 128 * c) for c in range(4)], axis=1)  # [128,4,N]
    return np.ascontiguousarray(m.astype(np.float16))


def _pm(a, chunks):
    """[chunks*128, F] -> partition-major [128, chunks, F] fp16, contiguous."""
    f = a.shape[-1]
    return np.ascontiguousarray(
        a.reshape(chunks, 128, f).transpose(1, 0, 2).astype(np.float16)
    )


def _pm_dc(a):
    """[1024, 512] weight chunk -> [128, 4(dc), 8(cc), 128] fp16."""
    # a[c, j]: c = 8 cc-chunks of 128 partitions; j = 4 dc-chunks of 128
    r = a.reshape(8, 128, 4, 128).transpose(1, 2, 0, 3)  # [128, dc, cc, 128]
    return np.ascontiguousarray(r.astype(np.float16))


def _make_in_maps(x, W_qkv, b_qkv, W_out):
    x = np.asarray(x, dtype=np.float32)
    W_qkv = np.asarray(W_qkv, dtype=np.float32)
    b_qkv = np.asarray(b_qkv, dtype=np.float32)
    W_out = np.asarray(W_out, dtype=np.float32)
    masks = _make_masks()
    xTr = [
        np.ascontiguousarray(
            _pm(x[b].T, 8).reshape(128, 8, 4, N).transpose(0, 2, 1, 3)
        )
        for b in range(B)
    ]
    in_maps = []
    for c in range(NCORES):
        b, g = c // G, c % G
        lo = CPH * g
        bqr = np.ascontiguousarray(
            b_qkv[lo : lo + CPH].reshape(4, 128).T.astype(np.float32)
        )
        bkr = np.ascontiguousarray(
            (SCALE * b_qkv[C + lo : C + lo + CPH]).reshape(4, 128).T.astype(np.float32)
        )
        in_maps.append(
            {
                "xTr": xTr[b],
                "wqr": _pm_dc(W_qkv[:, lo : lo + CPH]),
                "wkr": _pm_dc(W_qkv[:, C + lo : C + lo + CPH]),
                "wvr": _pm(W_qkv[:, 2 * C + lo : 2 * C + lo + CPH], 8),
                "bqr": bqr,
                "bkr": bkr,
                "wor": _pm(W_out[lo : lo + CPH, :], 4),
                "masks": masks,
            }
        )
    return in_maps


def _gather(results, b_out, bias_extra):
    bias = np.asarray(b_out, dtype=np.float32) + bias_extra
    out = np.empty((B, T, C), np.float32)
    for b in range(B):
        out[b] = (
            results[G * b]["yp"].astype(np.float32)
            + results[G * b + 1]["yp"].astype(np.float32)
            + bias[None, :]
        )
    return out


def kernel(x, W_qkv, b_qkv, W_out, b_out, **_):
    nc = _build_program()
    in_maps = _make_in_maps(x, W_qkv, b_qkv, W_out)
    res = bass_utils.run_bass_kernel_spmd(nc, in_maps, core_ids=list(range(NCORES)))
    bias_extra = np.asarray(b_qkv, np.float32)[2 * C :] @ np.asarray(W_out, np.float32)
    return _gather(res.results, b_out, bias_extra)


def kernel_traced(x, W_qkv, b_qkv, W_out, b_out, tmpdir=None, trace=True, **_):
    """Like kernel() but returns (out, exec_time_ns); used by test.py."""
    nc = _build_program()
    in_maps = _make_in_maps(x, W_qkv, b_qkv, W_out)
    res = bass_utils.run_bass_kernel_spmd(
        nc, in_maps, core_ids=list(range(NCORES)), trace=trace, tmpdir=tmpdir
    )
    bias_extra = np.asarray(b_qkv, np.float32)[2 * C :] @ np.asarray(W_out, np.float32)
    return _gather(res.results, b_out, bias_extra), res.exec_time_ns


# revision 8
# speedup vs baseline: 1.0559x; 1.0559x over previous
"""Causal self-attention (B=4, T=2048, C=1024, H=16, D=64) on 8 trn2 cores.

Sharding: core c -> (batch b = c//2, head-group g = c%2); a head group is
8 heads = 512 feature columns of each of Q/K/V.  Per core, one fully
software-pipelined program:

  - QKV projection blocks produce Q^T/K^T [64,2048] fp16 per head and
    V [2048,64] fp16 (+ a ones column that makes the AV matmul emit the
    softmax denominator for free).
  - Scores stay transposed (S^T[k,q]) so exp(S^T) feeds the AV matmul as
    the moving operand with no transposes anywhere.
  - Off-diagonal AV matmuls run in fp8e4 DoubleRow mode: the exp writes
    exp(s-1) straight to an fp8 pair tile (bias keeps values under the
    e4m3 max of 240 and cancels in the softmax), and one matmul contracts
    TWO key blocks at once against an interleaved fp8 copy of V.
    Diagonal blocks and all of qi=0 stay fp16.
  - The whole attention sweep is ONE flat software-pipelined stream over
    (qi, pr, kc) with the score matmul running one iteration ahead
    (across pr and qi boundaries), so the exp stream never waits for a
    pipeline restart.
  - Projection / out-projection matmuls are drip-fed between attention
    iterations at a per-qi rate; a dependency-driven need() forcing
    advances the drip exactly when an attention instruction requires a
    produced tile, so there are no bulk drains.
  - Engine balance: exp owns ACT; q/k bias-apply and tb0's V copies run
    on the otherwise-idle Scalar engine early on; the fp8 V copy runs on
    GpSimd (from SBUF); masks apply to both heads in one DVE op.

Host pre-arranges inputs partition-major (fp16; W_q/W_k dc-major so the
first dc chunk can DMA first) and sums the two per-batch partials,
folding b_out + b_v @ W_out (exact: softmax rows sum to 1).
"""

from collections import deque
from contextlib import ExitStack

import numpy as np

import concourse.bass as bass
import concourse.mybir as mybir
import concourse.tile as tile
from concourse import bacc
from concourse import bass_utils

F32 = mybir.dt.float32
F16 = mybir.dt.float16
F8 = mybir.dt.float8e4
EXPB = -1.0      # exp(s + EXPB): keeps fp8 et below e4m3 max (240); softmax-invariant
NF8PAIR = 6      # kc-block pairs 0..5 (blocks 0-11) get an fp8 copy of V

B, T, C = 4, 2048, 1024
H, D = 16, 64
G = 2            # head groups (cores per batch)
HPG = 8          # heads per group
CPH = HPG * D    # feature columns per group = 512
N = 512          # matmul moving free dim
NCORES = 8
SCALE = 1.0 / np.sqrt(D)
Exp = mybir.ActivationFunctionType.Exp
Ident = mybir.ActivationFunctionType.Identity

_CACHE = {}


def _build_program():
    if "nc" in _CACHE:
        return _CACHE["nc"]

    nc = bacc.Bacc("TRN2", target_bir_lowering=False, debug=False, num_devices=NCORES)

    # all inputs pre-arranged host-side: partition-major, fp16
    xTr = nc.dram_tensor("xTr", [128, 4, 8, N], F16, kind="ExternalInput").ap()
    wqr = nc.dram_tensor("wqr", [128, 4, 8, 128], F16, kind="ExternalInput").ap()
    wkr = nc.dram_tensor("wkr", [128, 4, 8, 128], F16, kind="ExternalInput").ap()
    wvr = nc.dram_tensor("wvr", [128, 8, CPH], F16, kind="ExternalInput").ap()
    bqr = nc.dram_tensor("bqr", [128, 4], F32, kind="ExternalInput").ap()
    bkr = nc.dram_tensor("bkr", [128, 4], F32, kind="ExternalInput").ap()  # ×SCALE
    wor = nc.dram_tensor("wor", [128, 4, C], F16, kind="ExternalInput").ap()
    masks = nc.dram_tensor("masks", [128, 4, N], F16, kind="ExternalInput").ap()
    yp = nc.dram_tensor("yp", [T, C], F16, kind="ExternalOutput").ap()

    with tile.TileContext(nc) as tc, ExitStack() as ctx:
        wpool = ctx.enter_context(tc.tile_pool(name="wpool", bufs=1))
        big = ctx.enter_context(tc.tile_pool(name="big", bufs=1))
        epool = ctx.enter_context(tc.tile_pool(name="et", bufs=6))
        e8pool = ctx.enter_context(tc.tile_pool(name="et8", bufs=3))
        mpool = ctx.enter_context(tc.tile_pool(name="mpool", bufs=4))
        blkps = ctx.enter_context(tc.tile_pool(name="blkps", bufs=2, space="PSUM"))
        sps = ctx.enter_context(tc.tile_pool(name="sps", bufs=2, space="PSUM"))
        avps = ctx.enter_context(tc.tile_pool(name="avps", bufs=1, space="PSUM"))

        XT = big.tile([128, 4, 8, N], F16)   # x^T resident (tb-major, c-chunks)
        QT = big.tile([128, 4, T], F16)   # Q^T (+bias)
        KT = big.tile([128, 4, T], F16)   # SCALE * (K^T + bias)
        VA = big.tile([128, 16, HPG, D + 1], F16)   # V rows + ones column
        # fp8 copy of V for off-diagonal AV: kc-block pairs interleaved for
        # DoubleRow ([partition, pair, j, head, d]; d padded to 66 so the
        # j-stride (8*66=528) is 16-byte aligned)
        VA8 = big.tile([128, NF8PAIR, 2, HPG, 66], F8)
        ON = big.tile([128, 4, T], F16)   # normalized O^T (c_in x tokens)

        WQ = wpool.tile([128, 4, 8, 128], F16)   # dc-major
        WK = wpool.tile([128, 4, 8, 128], F16)
        WV = wpool.tile([128, 8, CPH], F16)
        BQ = wpool.tile([128, 4], F32)
        BKs = wpool.tile([128, 4], F32)
        MS = wpool.tile([128, 4, N], F16)
        BEX = wpool.tile([128, 1], F32)   # exp bias (EXPB) as per-partition AP
        SCR = wpool.tile([128, N], F16)   # scratch for PE clock warm-up
        WO = wpool.tile([128, 4, C], F16)

        # PE clock warm-up: the HAM gate starts at 1.2GHz and needs ~3.4us
        # of sustained activity to release to 2.4GHz.  PE is idle until the
        # first weights land, so dummy matmuls on a memset scratch tile warm
        # it for free (outputs are never read).
        nc.any.memset(SCR[:], 0.0)
        wps = blkps.tile([128, N], F32, name="blk")
        for _ in range(9):
            nc.tensor.matmul(wps[:], SCR[:, 0:128], SCR[:], start=True, stop=True)

        # input DMAs ordered by first use: the first score pair only needs
        # WQ/WK dc0 + x tokens of tb0, so those stream first (x in c-chunk
        # pairs so q0's matmuls start before the whole tb lands)
        nc.sync.dma_start(BQ[:], bqr)
        nc.sync.dma_start(BKs[:], bkr)
        nc.sync.dma_start(WQ[:, 0], wqr[:, 0])
        for cc2 in range(4):
            sl = slice(2 * cc2, 2 * cc2 + 2)
            nc.sync.dma_start(XT[:, 0, sl, :], xTr[:, 0, sl, :])
        nc.sync.dma_start(WK[:, 0], wkr[:, 0])
        nc.sync.dma_start(MS[:], masks)
        for dc in range(1, 4):
            nc.sync.dma_start(WQ[:, dc], wqr[:, dc])
            nc.sync.dma_start(WK[:, dc], wkr[:, dc])
        nc.sync.dma_start(WV[:], wvr)
        nc.any.memset(VA[:, :, :, D : D + 1], 1.0)
        nc.any.memset(VA8[:, :, :, :, D : D + 1], 1.0)
        nc.any.memset(BEX[:], EXPB)
        for tb in range(1, 4):
            nc.sync.dma_start(XT[:, tb, :, :], xTr[:, tb, :, :])
        nc.sync.dma_start(WO[:], wor)

        def qkv_block_gen(tb, which, dc):
            """Generator: one projection block, yielding every 2 matmuls."""
            ps = blkps.tile([128, N], F32, name="blk")
            if which == "v":
                for cc in range(8):
                    nc.tensor.matmul(
                        ps[:],
                        XT[:, tb, cc, dc * 128 : (dc + 1) * 128],
                        WV[:, cc],
                        start=(cc == 0),
                        stop=(cc == 7),
                    )
                    if cc % 2 == 1:
                        yield
                bi = tb * 4 + dc
                src = ps[:].rearrange("p (h d) -> p h d", h=HPG)
                if tb == 0:
                    nc.scalar.copy(VA[:, bi, :, 0:D], src)
                else:
                    nc.vector.tensor_copy(VA[:, bi, :, 0:D], src)
                if bi < 2 * NF8PAIR:
                    # fp8 interleaved copy from the fp16 SBUF copy (2x
                    # packed DVE rate, cheaper than reading PSUM again)
                    nc.vector.tensor_copy(
                        VA8[:, bi // 2, bi % 2, :, 0:D], VA[:, bi, :, 0:D]
                    )
            else:
                WT, dst, scl, bias = (
                    (WQ, QT, 1.0, BQ) if which == "q" else (WK, KT, SCALE, BKs)
                )
                for cc in range(8):
                    nc.tensor.matmul(
                        ps[:],
                        WT[:, dc, cc, :],
                        XT[:, tb, cc, :],
                        start=(cc == 0),
                        stop=(cc == 7),
                    )
                    if cc % 2 == 1:
                        yield
                out_ap = dst[:, dc, tb * N : (tb + 1) * N]
                if tb <= 1:
                    # ACT is idle during the ramp / qi0: bias-apply there
                    nc.scalar.activation(
                        out_ap, ps[:], Ident, bias=bias[:, dc, None], scale=scl
                    )
                else:
                    nc.vector.scalar_tensor_tensor(
                        out=out_ap,
                        in0=ps[:],
                        scalar=scl,
                        in1=bias[:, dc, None].to_broadcast((128, N)),
                        op0=mybir.AluOpType.mult,
                        op1=mybir.AluOpType.add,
                    )

        def y_block_gen(ic, ob, alt=False):
            """Generator: one out-projection block, yielding every 2 matmuls.

            Tail blocks (alt=True) borrow the first bank of the (by then
            idle) score psum tiles so four blocks can rotate in flight."""
            if alt:
                ypt = sps.tile([128, 2 * N], F32, name="sp")[:, 0:N]
            else:
                ypt = blkps.tile([128, N], F32, name="blk")
            for cc4 in range(4):
                nc.tensor.matmul(
                    ypt[:],
                    ON[:, cc4, ic * 128 : (ic + 1) * 128],
                    WO[:, cc4, ob * N : (ob + 1) * N],
                    start=(cc4 == 0),
                    stop=(cc4 == 3),
                )
                if cc4 % 2 == 1:
                    yield
            ysb = mpool.tile([128, N], F16, name="ysb")
            nc.vector.tensor_copy(ysb[:], ypt[:])
            nc.sync.dma_start(
                yp[ic * 128 : (ic + 1) * 128, ob * N : (ob + 1) * N], ysb[:]
            )

        def warm_gen(n):
            # keeps the HAM clock gate warm across DMA-paced prefix gaps
            wp = blkps.tile([128, N], F32, name="blk")
            for _ in range(n):
                nc.tensor.matmul(
                    wp[:], SCR[:, 0:128], SCR[:], start=True, stop=True
                )
                yield

        # ---- drip machinery: FIFO generator queue with labels + need()
        drip = deque()           # (label, generator)
        done_labels = set()

        def drip_advance(n):
            for _ in range(n):
                while drip:
                    lbl, g = drip[0]
                    try:
                        next(g)
                        break
                    except StopIteration:
                        if lbl is not None:
                            done_labels.add(lbl)
                        drip.popleft()
                else:
                    return

        def need(lbl):
            while lbl not in done_labels:
                if not drip:
                    raise RuntimeError(f"need({lbl}) but drip empty")
                drip_advance(1)

        def insert_before(lbl, items):
            """Insert (label, gen) items before the (unstarted) entry with
            label `lbl`, keeping FIFO advance order = allocation order."""
            lst = list(drip)
            for pos, (l, _) in enumerate(lst):
                if l == lbl:
                    break
            else:
                pos = len(lst)
            drip.clear()
            drip.extend(lst[:pos] + items + lst[pos:])

        def add(which, tb, dc, warm=0):
            drip.append(((which, tb, dc), qkv_block_gen(tb, which, dc)))
            if warm:
                drip.append((None, warm_gen(warm)))

        # projection generators in due-date order; within tb t the first
        # score pair needs q-dc0 (at qi=t start) then k-dc0 / v-dc* (at
        # pr0's kc=4t), then q/k per pr
        add("q", 0, 0, warm=3)
        add("k", 0, 0, warm=3)
        for dc in range(4):
            add("v", 0, dc, warm=(3 if dc < 2 else 0))
        for dc in range(1, 4):
            add("q", 0, dc)
            add("k", 0, dc)
        for t in range(1, 4):
            add("q", t, 0)
            add("k", t, 0)
            for dc in range(4):
                add("v", t, dc)
            for dc in range(1, 4):
                add("q", t, dc)
                add("k", t, dc)

        # out-projection blocks and the explicit tail (last 8 blocks)
        tail_blocks = [(12 + i4, ob) for i4 in range(4) for ob in range(2)]
        tail_pre = {}

        def tail_prefix_gen(bi, ic, ob):
            # pre-issued during qi=3's last iterations: only needs ON
            # pr0-2, which are ready well before pr3 finishes
            ypt = blkps.tile([128, N], F32, name="blk")
            for cc4 in range(3):
                nc.tensor.matmul(
                    ypt[:],
                    ON[:, cc4, ic * 128 : (ic + 1) * 128],
                    WO[:, cc4, ob * N : (ob + 1) * N],
                    start=(cc4 == 0),
                    stop=False,
                )
                yield
            tail_pre[bi] = ypt[:]

        # ---- flat attention stream -------------------------------------
        iters = [
            (qi, pr, kc)
            for qi in range(4)
            for pr in range(4)
            for kc in range(4 * qi + 4)
        ]
        NIT = len(iters)

        def emit_s(qi, pr, kc):
            # both heads' score tiles in one 2-bank psum tile so the exp
            # runs 1024 wide; the two matmuls run concurrently (row groups
            # 0-1 / 2-3).  Diagonal chunks only compute the causally-
            # reachable column range [vq:].
            need(("q", qi, pr))
            need(("k", kc // 4, pr))
            vq = max(0, (kc - 4 * qi) * 128)
            sp = sps.tile([128, 2 * N], F32, name="sp")
            for hi in range(2):
                off = 64 * hi
                nc.tensor.matmul(
                    sp[:, hi * N + vq : (hi + 1) * N],
                    KT[off : off + 64, pr, kc * 128 : (kc + 1) * 128],
                    QT[off : off + 64, pr, qi * N + vq : (qi + 1) * N],
                    start=True,
                    stop=True,
                )
            return sp

        # drip rate per attention iteration (num/den) by qi, tuned so each
        # token block's projections finish just before they are needed and
        # the out-projection backlog lands in qi2/qi3 ACT slack
        RATE = {0: (4, 1), 1: (3, 2), 2: (3, 2), 3: (3, 2)}

        sp_tiles = {0: emit_s(*iters[0]), 1: emit_s(*iters[1])}
        avs = None
        bacc_ctr = 0
        for idx, (qi, pr, kc) in enumerate(iters):
            nkc = 4 * qi + 4
            if kc == 0:
                avs = [avps.tile([D + 1, N], F32, name=f"av{hi}") for hi in range(2)]
                if qi == 3 and pr == 0:
                    # out-projection backlog for qi2 (its ON rows are done)
                    for i4 in range(4):
                        for ob in range(2):
                            drip.append((None, y_block_gen(8 + i4, ob)))
                if qi == 3 and pr == 3:
                    # tail prefixes read ON pr0-2 of qi3: all normalizes
                    # they touch are emitted by now
                    for bi in (0, 2):
                        drip.append((None, tail_prefix_gen(bi, *tail_blocks[bi])))
            sp_cur = sp_tiles.pop(idx)

            # the exp runs FIRST so the score stream can stay two
            # iterations ahead (the pool's write-after-read dep on the sp
            # buffer needs this exp emitted before S(idx+2) reuses it)
            vq = max(0, (kc - 4 * qi) * 128)
            nfp8 = 4 * qi
            fp8 = kc < nfp8
            if fp8:
                # fp8 path: exp(s-1) into the pair tile; the odd member
                # fires one DoubleRow AV per head contracting both blocks
                if kc % 2 == 0:
                    et2 = e8pool.tile([128, 2, 2, N], F8, name="et2")
                nc.scalar.activation(
                    et2[:, kc % 2, :, :], sp_cur[:], Exp, bias=BEX[:]
                )
            else:
                et = epool.tile([128, 2 * N], F16, name="et")
                if vq == 0:
                    nc.scalar.activation(et[:], sp_cur[:], Exp, bias=BEX[:])
                else:
                    # both heads' [vq:] ranges in one strided-AP
                    # instruction (saves the per-instr ACT overhead)
                    nc.scalar.activation(
                        et[:].rearrange("p (h q) -> p h q", h=2)[:, :, vq:],
                        sp_cur[:].rearrange("p (h q) -> p h q", h=2)[:, :, vq:],
                        Exp,
                        bias=BEX[:],
                    )
                if kc >= 4 * qi:
                    # mask only the 128-wide diagonal sub-block for both
                    # heads in one strided op; the rest of [vq:] is fully
                    # below the diagonal
                    nc.vector.tensor_tensor(
                        et[:].rearrange("p (h q) -> p h q", h=2)[
                            :, :, vq : vq + 128
                        ],
                        et[:].rearrange("p (h q) -> p h q", h=2)[
                            :, :, vq : vq + 128
                        ],
                        MS[:, kc - 4 * qi, None, vq : vq + 128].to_broadcast(
                            (128, 2, 128)
                        ),
                        mybir.AluOpType.mult,
                    )

            if idx + 2 < NIT:
                sp_tiles[idx + 2] = emit_s(*iters[idx + 2])

            if fp8:
                if kc % 2 == 1:
                    need(("v", kc // 4, kc % 4))
                    need(("v", (kc - 1) // 4, (kc - 1) % 4))
                    for hi in range(2):
                        nc.tensor.matmul(
                            avs[hi][:],
                            VA8[:, kc // 2, :, 2 * pr + hi, 0 : D + 1],
                            et2[:, :, hi, :],
                            start=(kc == 1),
                            stop=False,
                            perf_mode=mybir.MatmulPerfMode.DoubleRow,
                        )
            else:
                need(("v", kc // 4, kc % 4))
                for hi in range(2):
                    nc.tensor.matmul(
                        avs[hi][:, vq:],
                        VA[:, kc, 2 * pr + hi, :],
                        et[:, hi * N + vq : (hi + 1) * N],
                        start=(kc == 0),
                        stop=(kc == nkc - 1),
                    )
            bnum, bden = RATE[qi]
            bacc_ctr += bnum
            drip_advance(bacc_ctr // bden)
            bacc_ctr %= bden

            if kc == nkc - 1:
                last = qi == 3 and pr == 3
                if last:
                    # tail: read PSUM directly (no later user of the banks)
                    # and interleave the two heads' chains to cut latency
                    dns, rbs, rcs = [], [], []
                    for hi in range(2):
                        dn = mpool.tile([1, N], F32, name="dn")
                        nc.vector.tensor_copy(dn[:], avs[hi][D : D + 1, :])
                        dns.append(dn)
                    for hi in range(2):
                        rb = mpool.tile([64, N], F32, name="rb")
                        nc.gpsimd.partition_broadcast(rb[:], dns[hi][:])
                        rbs.append(rb)
                    for hi in range(2):
                        rc = mpool.tile([64, N], F32, name="rc")
                        nc.vector.reciprocal_approx_fast(rc[:], rbs[hi][:])
                        rcs.append(rc)
                    for hi in range(2):
                        off = 64 * hi
                        seg = ON[off : off + 64, pr, qi * N : (qi + 1) * N]
                        nc.vector.tensor_tensor(
                            seg, avs[hi][0:64, :], rcs[hi][:], mybir.AluOpType.mult
                        )
                else:
                    for hi in range(2):
                        off = 64 * hi
                        # one copy releases the accumulator bank; the rest
                        # of the normalize chain runs off SBUF, off the
                        # critical path
                        oc = mpool.tile([D + 1, N], F32, name="oc")
                        nc.vector.tensor_copy(oc[:], avs[hi][:])
                        dn = mpool.tile([1, N], F32, name="dn")
                        nc.vector.tensor_copy(dn[:], oc[D : D + 1, :])
                        rb = mpool.tile([64, N], F32, name="rb")
                        nc.gpsimd.partition_broadcast(rb[:], dn[:])
                        rc = mpool.tile([64, N], F32, name="rc")
                        nc.vector.reciprocal_approx_fast(rc[:], rb[:])
                        seg = ON[off : off + 64, pr, qi * N : (qi + 1) * N]
                        nc.vector.tensor_tensor(
                            seg, oc[0:64, :], rc[:], mybir.AluOpType.mult
                        )
                # out-projection blocks become available at each qi end;
                # insert them in due-date order (before tb3's generators,
                # whose own dues are enforced by need()) so they drip
                # during qi2/qi3 ACT slack instead of piling up at the end
                if pr == 3 and qi < 2:
                    blocks = [
                        (None, y_block_gen(4 * qi + i4, ob))
                        for i4 in range(4)
                        for ob in range(2)
                    ]
                    insert_before(("q", 3, qi), blocks)
        while drip:
            drip_advance(1)

        # tail: the last query block's out-projections.  Four blocks' first
        # three contraction chunks (they only need ON pr0-2) issue
        # back-to-back across all four free psum slots, so PE has work
        # queued while the last normalize chain produces ON pr3; their
        # final chunks then land in quick succession.  PSUM->SBUF copies
        # run on the by-now idle Scalar engine.
        def tail_psum(bi):
            if bi % 2 == 1:
                return sps.tile([128, 2 * N], F32, name="sp")[:, 0:N]
            return blkps.tile([128, N], F32, name="blk")

        def tail_finish(ypt, ic, ob):
            nc.tensor.matmul(
                ypt,
                ON[:, 3, ic * 128 : (ic + 1) * 128],
                WO[:, 3, ob * N : (ob + 1) * N],
                start=False,
                stop=True,
            )
            ysb = mpool.tile([128, N], F16, name="ysb")
            nc.scalar.copy(ysb[:], ypt)
            nc.sync.dma_start(
                yp[ic * 128 : (ic + 1) * 128, ob * N : (ob + 1) * N], ysb[:]
            )

        pending = []
        for bi, (ic, ob) in enumerate(tail_blocks):
            if bi in tail_pre:
                pending.append((tail_pre[bi], ic, ob))
            else:
                ypt = tail_psum(bi)
                for cc4 in range(3):
                    nc.tensor.matmul(
                        ypt,
                        ON[:, cc4, ic * 128 : (ic + 1) * 128],
                        WO[:, cc4, ob * N : (ob + 1) * N],
                        start=(cc4 == 0),
                        stop=False,
                    )
                pending.append((ypt, ic, ob))
            if bi >= 3:
                # keep at most 4 prefixes in flight (4 psum slots)
                tail_finish(*pending.pop(0))
        for args in pending:
            tail_finish(*args)

    nc.compile()
    _CACHE["nc"] = nc
    return nc


def _make_masks():
    kp = np.arange(128)[:, None]
    qf = np.arange(N)[None, :]
    m = np.stack([(qf >= kp + 128 * c) for c in range(4)], axis=1)  # [128,4,N]
    return np.ascontiguousarray(m.astype(np.float16))


def _pm(a, chunks):
    """[chunks*128, F] -> partition-major [128, chunks, F] fp16, contiguous."""
    f = a.shape[-1]
    return np.ascontiguousarray(
        a.reshape(chunks, 128, f).transpose(1, 0, 2).astype(np.float16)
    )


def _pm_dc(a):
    """[1024, 512] weight chunk -> [128, 4(dc), 8(cc), 128] fp16."""
    # a[c, j]: c = 8 cc-chunks of 128 partitions; j = 4 dc-chunks of 128
    r = a.reshape(8, 128, 4, 128).transpose(1, 2, 0, 3)  # [128, dc, cc, 128]
    return np.ascontiguousarray(r.astype(np.float16))


def _make_in_maps(x, W_qkv, b_qkv, W_out):
    x = np.asarray(x, dtype=np.float32)
    W_qkv = np.asarray(W_qkv, dtype=np.float32)
    b_qkv = np.asarray(b_qkv, dtype=np.float32)
    W_out = np.asarray(W_out, dtype=np.float32)
    masks = _make_masks()
    xTr = [
        np.ascontiguousarray(
            _pm(x[b].T, 8).reshape(128, 8, 4, N).transpose(0, 2, 1, 3)
        )
        for b in range(B)
    ]
    in_maps = []
    for c in range(NCORES):
        b, g = c // G, c % G
        lo = CPH * g
        bqr = np.ascontiguousarray(
            b_qkv[lo : lo + CPH].reshape(4, 128).T.astype(np.float32)
        )
        bkr = np.ascontiguousarray(
            (SCALE * b_qkv[C + lo : C + lo + CPH]).reshape(4, 128).T.astype(np.float32)
        )
        in_maps.append(
            {
                "xTr": xTr[b],
                "wqr": _pm_dc(W_qkv[:, lo : lo + CPH]),
                "wkr": _pm_dc(W_qkv[:, C + lo : C + lo + CPH]),
                "wvr": _pm(W_qkv[:, 2 * C + lo : 2 * C + lo + CPH], 8),
                "bqr": bqr,
                "bkr": bkr,
                "wor": _pm(W_out[lo : lo + CPH, :], 4),
                "masks": masks,
            }
        )
    return in_maps


def _gather(results, b_out, bias_extra):
    bias = np.asarray(b_out, dtype=np.float32) + bias_extra
    out = np.empty((B, T, C), np.float32)
    for b in range(B):
        out[b] = (
            results[G * b]["yp"].astype(np.float32)
            + results[G * b + 1]["yp"].astype(np.float32)
            + bias[None, :]
        )
    return out


def kernel(x, W_qkv, b_qkv, W_out, b_out, **_):
    nc = _build_program()
    in_maps = _make_in_maps(x, W_qkv, b_qkv, W_out)
    res = bass_utils.run_bass_kernel_spmd(nc, in_maps, core_ids=list(range(NCORES)))
    bias_extra = np.asarray(b_qkv, np.float32)[2 * C :] @ np.asarray(W_out, np.float32)
    return _gather(res.results, b_out, bias_extra)


def kernel_traced(x, W_qkv, b_qkv, W_out, b_out, tmpdir=None, trace=True, **_):
    """Like kernel() but returns (out, exec_time_ns); used by test.py."""
    nc = _build_program()
    in_maps = _make_in_maps(x, W_qkv, b_qkv, W_out)
    res = bass_utils.run_bass_kernel_spmd(
        nc, in_maps, core_ids=list(range(NCORES)), trace=trace, tmpdir=tmpdir
    )
    bias_extra = np.asarray(b_qkv, np.float32)[2 * C :] @ np.asarray(W_out, np.float32)
    return _gather(res.results, b_out, bias_extra), res.exec_time_ns


# revision 19
# speedup vs baseline: 1.0964x; 1.0383x over previous
"""Causal self-attention (B=4, T=2048, C=1024, H=16, D=64) on 8 trn2 cores.

Sharding: core c -> (batch b = c//2, head-group g = c%2); a head group is
8 heads = 512 feature columns of each of Q/K/V.  Per core, one fully
software-pipelined program:

  - QKV projection blocks produce Q^T/K^T [64,2048] fp16 per head and
    V [2048,64] fp16 (+ a ones column that makes the AV matmul emit the
    softmax denominator for free).
  - Scores stay transposed (S^T[k,q]) so exp(S^T) feeds the AV matmul as
    the moving operand with no transposes anywhere.
  - Off-diagonal AV matmuls run in fp8e4 DoubleRow mode: the exp writes
    exp(s-1) straight to an fp8 pair tile (bias keeps values under the
    e4m3 max of 240 and cancels in the softmax), and one matmul contracts
    TWO key blocks at once against an interleaved fp8 copy of V.
    Diagonal blocks and all of qi=0 stay fp16.
  - The whole attention sweep is ONE flat software-pipelined stream over
    (qi, pr, kc) with the score matmul running one iteration ahead
    (across pr and qi boundaries), so the exp stream never waits for a
    pipeline restart.
  - Projection / out-projection matmuls are drip-fed between attention
    iterations at a per-qi rate; a dependency-driven need() forcing
    advances the drip exactly when an attention instruction requires a
    produced tile, so there are no bulk drains.
  - Engine balance: exp owns ACT; q/k bias-apply and tb0's V copies run
    on the otherwise-idle Scalar engine early on; the fp8 V copy runs on
    GpSimd (from SBUF); masks apply to both heads in one DVE op.

Host pre-arranges inputs partition-major (fp16; W_q/W_k dc-major so the
first dc chunk can DMA first) and sums the two per-batch partials,
folding b_out + b_v @ W_out (exact: softmax rows sum to 1).
"""

from collections import deque
from contextlib import ExitStack

import numpy as np

import concourse.bass as bass
import concourse.mybir as mybir
import concourse.tile as tile
from concourse import bacc
from concourse import bass_utils

F32 = mybir.dt.float32
F16 = mybir.dt.float16
F8 = mybir.dt.float8e4
EXPB = -1.0      # exp(s + EXPB): keeps fp8 et below e4m3 max (240); softmax-invariant
NF8PAIR = 6      # kc-block pairs 0..5 (blocks 0-11) get an fp8 copy of V

B, T, C = 4, 2048, 1024
H, D = 16, 64
G = 2            # head groups (cores per batch)
HPG = 8          # heads per group
CPH = HPG * D    # feature columns per group = 512
N = 512          # matmul moving free dim
NCORES = 8
SCALE = 1.0 / np.sqrt(D)
Exp = mybir.ActivationFunctionType.Exp
Ident = mybir.ActivationFunctionType.Identity

_CACHE = {}


def _build_program():
    if "nc" in _CACHE:
        return _CACHE["nc"]

    nc = bacc.Bacc("TRN2", target_bir_lowering=False, debug=False, num_devices=NCORES)

    # all inputs pre-arranged host-side: partition-major, fp16
    xTr = nc.dram_tensor("xTr", [128, 4, 8, N], F16, kind="ExternalInput").ap()
    wqr = nc.dram_tensor("wqr", [128, 4, 8, 128], F16, kind="ExternalInput").ap()
    wkr = nc.dram_tensor("wkr", [128, 4, 8, 128], F16, kind="ExternalInput").ap()
    wvr = nc.dram_tensor("wvr", [128, 8, CPH], F16, kind="ExternalInput").ap()
    bqr = nc.dram_tensor("bqr", [128, 4], F32, kind="ExternalInput").ap()
    bkr = nc.dram_tensor("bkr", [128, 4], F32, kind="ExternalInput").ap()  # ×SCALE
    wor = nc.dram_tensor("wor", [128, 4, C], F16, kind="ExternalInput").ap()
    masks = nc.dram_tensor("masks", [128, 4, N], F16, kind="ExternalInput").ap()
    yp = nc.dram_tensor("yp", [T, C], F16, kind="ExternalOutput").ap()

    with tile.TileContext(nc) as tc, ExitStack() as ctx:
        wpool = ctx.enter_context(tc.tile_pool(name="wpool", bufs=1))
        big = ctx.enter_context(tc.tile_pool(name="big", bufs=1))
        epool = ctx.enter_context(tc.tile_pool(name="et", bufs=8))
        e8pool = ctx.enter_context(tc.tile_pool(name="et8", bufs=4))
        mpool = ctx.enter_context(tc.tile_pool(name="mpool", bufs=4))
        blkps = ctx.enter_context(tc.tile_pool(name="blkps", bufs=2, space="PSUM"))
        sps = ctx.enter_context(tc.tile_pool(name="sps", bufs=2, space="PSUM"))
        avps = ctx.enter_context(tc.tile_pool(name="avps", bufs=1, space="PSUM"))

        XT = big.tile([128, 4, 8, N], F16)   # x^T resident (tb-major, c-chunks)
        QT = big.tile([128, 4, T], F16)   # Q^T (+bias)
        KT = big.tile([128, 4, T], F16)   # SCALE * (K^T + bias)
        VA = big.tile([128, 16, HPG, D + 1], F16)   # V rows + ones column
        # fp8 copy of V for off-diagonal AV: kc-block pairs interleaved for
        # DoubleRow ([partition, pair, j, head, d]; d padded to 66 so the
        # j-stride (8*66=528) is 16-byte aligned)
        VA8 = big.tile([128, NF8PAIR, 2, HPG, 66], F8)
        ON = big.tile([128, 4, T], F16)   # normalized O^T (c_in x tokens)

        WQ = wpool.tile([128, 4, 8, 128], F16)   # dc-major
        WK = wpool.tile([128, 4, 8, 128], F16)
        WV = wpool.tile([128, 8, CPH], F16)
        BQ = wpool.tile([128, 4], F32)
        BKs = wpool.tile([128, 4], F32)
        MS = wpool.tile([128, 4, N], F16)
        BEX = wpool.tile([128, 1], F32)   # exp bias (EXPB) as per-partition AP
        SCR = wpool.tile([128, N], F16)   # scratch for PE clock warm-up
        WO = wpool.tile([128, 4, C], F16)

        # PE clock warm-up: the HAM gate starts at 1.2GHz and needs ~3.4us
        # of sustained activity to release to 2.4GHz.  PE is idle until the
        # first weights land, so dummy matmuls on a memset scratch tile warm
        # it for free (outputs are never read).
        nc.any.memset(SCR[:], 0.0)
        wps = blkps.tile([128, N], F32, name="blk")
        for _ in range(16):
            nc.tensor.matmul(wps[:], SCR[:, 0:128], SCR[:], start=True, stop=True)

        # input DMAs ordered by first use: the first score pair only needs
        # WQ/WK dc0 + x tokens of tb0, so those stream first (x in c-chunk
        # pairs so q0's matmuls start before the whole tb lands)
        nc.sync.dma_start(BQ[:], bqr)
        nc.sync.dma_start(BKs[:], bkr)
        nc.sync.dma_start(WQ[:, 0], wqr[:, 0])
        for cc2 in range(4):
            sl = slice(2 * cc2, 2 * cc2 + 2)
            nc.sync.dma_start(XT[:, 0, sl, :], xTr[:, 0, sl, :])
        nc.sync.dma_start(WK[:, 0], wkr[:, 0])
        nc.sync.dma_start(MS[:], masks)
        for cc2 in range(4):
            sl = slice(2 * cc2, 2 * cc2 + 2)
            nc.sync.dma_start(WV[:, sl], wvr[:, sl])
        for dc in range(1, 4):
            nc.sync.dma_start(WQ[:, dc], wqr[:, dc])
            nc.sync.dma_start(WK[:, dc], wkr[:, dc])
        nc.any.memset(VA[:, :, :, D : D + 1], 1.0)
        nc.any.memset(VA8[:, :, :, :, D : D + 1], 1.0)
        nc.any.memset(BEX[:], EXPB)
        for tb in range(1, 4):
            nc.sync.dma_start(XT[:, tb, :, :], xTr[:, tb, :, :])
        nc.sync.dma_start(WO[:], wor)

        def qkv_block_gen(tb, which, dc):
            """Generator: one projection block, yielding every matmul."""
            ps = blkps.tile([128, N], F32, name="blk")
            if which == "v":
                for cc in range(8):
                    nc.tensor.matmul(
                        ps[:],
                        XT[:, tb, cc, dc * 128 : (dc + 1) * 128],
                        WV[:, cc],
                        start=(cc == 0),
                        stop=(cc == 7),
                    )
                    yield
                bi = tb * 4 + dc
                src = ps[:].rearrange("p (h d) -> p h d", h=HPG)
                nc.vector.tensor_copy(VA[:, bi, :, 0:D], src)
                if bi < 2 * NF8PAIR:
                    # fp8 interleaved copy from the fp16 SBUF copy (2x
                    # packed DVE rate, cheaper than reading PSUM again)
                    nc.vector.tensor_copy(
                        VA8[:, bi // 2, bi % 2, :, 0:D], VA[:, bi, :, 0:D]
                    )
            else:
                WT, dst, scl, bias = (
                    (WQ, QT, 1.0, BQ) if which == "q" else (WK, KT, SCALE, BKs)
                )
                for cc in range(8):
                    nc.tensor.matmul(
                        ps[:],
                        WT[:, dc, cc, :],
                        XT[:, tb, cc, :],
                        start=(cc == 0),
                        stop=(cc == 7),
                    )
                    yield
                out_ap = dst[:, dc, tb * N : (tb + 1) * N]
                if tb <= 1:
                    # ACT is idle during the ramp / qi0: bias-apply there
                    nc.scalar.activation(
                        out_ap, ps[:], Ident, bias=bias[:, dc, None], scale=scl
                    )
                else:
                    nc.vector.scalar_tensor_tensor(
                        out=out_ap,
                        in0=ps[:],
                        scalar=scl,
                        in1=bias[:, dc, None].to_broadcast((128, N)),
                        op0=mybir.AluOpType.mult,
                        op1=mybir.AluOpType.add,
                    )

        def y_block_gen(ic, ob, alt=False):
            """Generator: one out-projection block, yielding every 2 matmuls.

            Tail blocks (alt=True) borrow the first bank of the (by then
            idle) score psum tiles so four blocks can rotate in flight."""
            if alt:
                ypt = sps.tile([128, 2 * N], F32, name="sp")[:, 0:N]
            else:
                ypt = blkps.tile([128, N], F32, name="blk")
            for cc4 in range(4):
                nc.tensor.matmul(
                    ypt[:],
                    ON[:, cc4, ic * 128 : (ic + 1) * 128],
                    WO[:, cc4, ob * N : (ob + 1) * N],
                    start=(cc4 == 0),
                    stop=(cc4 == 3),
                )
                yield
            ysb = mpool.tile([128, N], F16, name="ysb")
            nc.vector.tensor_copy(ysb[:], ypt[:])
            nc.sync.dma_start(
                yp[ic * 128 : (ic + 1) * 128, ob * N : (ob + 1) * N], ysb[:]
            )

        def warm_gen(n):
            # keeps the HAM clock gate warm across DMA-paced prefix gaps
            wp = blkps.tile([128, N], F32, name="blk")
            for _ in range(n):
                nc.tensor.matmul(
                    wp[:], SCR[:, 0:128], SCR[:], start=True, stop=True
                )
                yield

        # out-projection blocks and the explicit tail (last 8 blocks)
        tail_blocks = [(12 + i4, ob) for i4 in range(4) for ob in range(2)]
        tail_pre = {}

        def tail_prefix_gen(bi, ic, ob):
            # pre-issued during qi=3's last iterations: only needs ON
            # pr0-2, which are ready well before pr3 finishes
            ypt = blkps.tile([128, N], F32, name="blk")
            for cc4 in range(3):
                nc.tensor.matmul(
                    ypt[:],
                    ON[:, cc4, ic * 128 : (ic + 1) * 128],
                    WO[:, cc4, ob * N : (ob + 1) * N],
                    start=(cc4 == 0),
                    stop=False,
                )
                yield
            tail_pre[bi] = ypt[:]

        # ---- flat attention stream -------------------------------------
        iters = [
            (qi, pr, kc)
            for qi in range(4)
            for pr in range(4)
            for kc in range(4 * qi + 4)
        ]
        NIT = len(iters)

        # first iteration each produced tile is needed at (S runs 2 ahead)
        first_need = {}

        def _reg(lbl, it):
            if lbl not in first_need:
                first_need[lbl] = it

        for j, (qi, pr, kc) in enumerate(iters):
            se = max(0, j - 2)
            _reg(("q", qi, pr), se)
            _reg(("k", kc // 4, pr), se)
            if kc < 4 * qi:
                if kc % 2 == 1:
                    _reg(("v", kc // 4, kc % 4), j)
                    _reg(("v", (kc - 1) // 4, (kc - 1) % 4), j)
            else:
                _reg(("v", kc // 4, kc % 4), j)

        # ---- drip machinery: FIFO generator plan with labels, release
        # dates (gens reading ON may not start before their inputs are
        # emitted) and an EDF-smoothed advance schedule (1 matmul per
        # advance, so forced catch-ups never form multi-us PE bursts)
        plan = []  # (label, gen, n_advances, due_iter, release_iter)

        def padd(which, tb, dc):
            lbl = (which, tb, dc)
            plan.append((lbl, qkv_block_gen(tb, which, dc), 8, first_need[lbl], 0))

        def pwarm(n):
            plan.append((None, warm_gen(n), n, 0, 0))

        pwarm(6)
        padd("q", 0, 0)
        pwarm(3)
        padd("k", 0, 0)
        pwarm(3)
        for dc in range(4):
            padd("v", 0, dc)
        for dc in range(1, 4):
            padd("q", 0, dc)
            padd("k", 0, dc)
        for t in range(1, 3):
            padd("q", t, 0)
            padd("k", t, 0)
            for dc in range(4):
                padd("v", t, dc)
            for dc in range(1, 4):
                padd("q", t, dc)
                padd("k", t, dc)
        # qi0/qi1 out-projection blocks: released at the end of their qi,
        # due before the respective ACT-slack window closes
        yrel = {0: 16, 1: 48, 2: 96}
        ydue = {0: 95, 1: 130, 2: NIT - 1}
        for i4 in range(4):
            for ob in range(2):
                plan.append(
                    (None, y_block_gen(0 + i4, ob), 4, ydue[0], yrel[0])
                )
        padd("q", 3, 0)
        for i4 in range(4):
            for ob in range(2):
                plan.append(
                    (None, y_block_gen(4 + i4, ob), 4, ydue[1], yrel[1])
                )
        padd("k", 3, 0)
        for dc in range(4):
            padd("v", 3, dc)
        for dc in range(1, 4):
            padd("q", 3, dc)
            padd("k", 3, dc)
        for i4 in range(4):
            for ob in range(2):
                plan.append(
                    (None, y_block_gen(8 + i4, ob), 4, ydue[2], yrel[2])
                )
        for bi in (0, 2):
            plan.append(
                (None, tail_prefix_gen(bi, *tail_blocks[bi]), 3, NIT - 1, 144)
            )

        # EDF schedule: per-iteration advance counts meeting every due
        # date at the smoothest possible rate, capped by each qi's PE
        # slack per exp
        CAP = {0: 6.0, 1: 2.0, 2: 2.1, 3: 2.2}
        cum = 0
        centries = []  # (cum_advances, due, release)
        for lbl, _, nadv, due, rel in plan:
            cum += nadv
            centries.append((cum, due, rel))
        sched = [0] * NIT
        Df = 0.0
        emitted = 0
        for i in range(NIT):
            qi_i = iters[i][0]
            barrier = 0
            for cu, du, rel in centries:
                if rel > i:
                    break
                barrier = cu
            needed = 0.0
            forced = 0.0
            for cu, du, rel in centries:
                if cu <= Df:
                    continue
                if du <= i + 1:
                    forced = max(forced, cu)
                elif du > i:
                    needed = max(needed, (cu - Df) / (du - i))
            Df = min(barrier, max(Df + min(CAP[qi_i], needed), forced))
            sched[i] = int(Df) - emitted
            emitted = int(Df)

        drip = deque(plan)
        done_labels = set()
        cur_it = [0]

        def drip_advance(n):
            for _ in range(n):
                while drip:
                    lbl, g, _, _, rel = drip[0]
                    if rel > cur_it[0]:
                        return
                    try:
                        next(g)
                        break
                    except StopIteration:
                        if lbl is not None:
                            done_labels.add(lbl)
                        drip.popleft()
                else:
                    return

        def need(lbl):
            while lbl not in done_labels:
                if not drip:
                    raise RuntimeError(f"need({lbl}) but drip empty")
                if drip[0][4] > cur_it[0]:
                    raise RuntimeError(f"need({lbl}) blocked by release")
                drip_advance(1)

        def emit_s(qi, pr, kc):
            # both heads' score tiles in one 2-bank psum tile so the exp
            # runs 1024 wide; the two matmuls run concurrently (row groups
            # 0-1 / 2-3).  Diagonal chunks only compute the causally-
            # reachable column range [vq:].
            need(("q", qi, pr))
            need(("k", kc // 4, pr))
            vq = max(0, (kc - 4 * qi) * 128)
            sp = sps.tile([128, 2 * N], F32, name="sp")
            for hi in range(2):
                off = 64 * hi
                nc.tensor.matmul(
                    sp[:, hi * N + vq : (hi + 1) * N],
                    KT[off : off + 64, pr, kc * 128 : (kc + 1) * 128],
                    QT[off : off + 64, pr, qi * N + vq : (qi + 1) * N],
                    start=True,
                    stop=True,
                )
            return sp

        sp_tiles = {0: emit_s(*iters[0]), 1: emit_s(*iters[1])}
        avs = None
        for idx, (qi, pr, kc) in enumerate(iters):
            cur_it[0] = idx
            nkc = 4 * qi + 4
            if kc == 0:
                avs = [avps.tile([D + 1, N], F32, name=f"av{hi}") for hi in range(2)]
            sp_cur = sp_tiles.pop(idx)

            # the exp runs FIRST so the score stream can stay two
            # iterations ahead (the pool's write-after-read dep on the sp
            # buffer needs this exp emitted before S(idx+2) reuses it)
            vq = max(0, (kc - 4 * qi) * 128)
            nfp8 = 4 * qi
            fp8 = kc < nfp8
            if fp8:
                # fp8 path: exp(s-1) into the pair tile; the odd member
                # fires one DoubleRow AV per head contracting both blocks
                if kc % 2 == 0:
                    et2 = e8pool.tile([128, 2, 2, N], F8, name="et2")
                nc.scalar.activation(
                    et2[:, kc % 2, :, :], sp_cur[:], Exp, bias=BEX[:]
                )
            else:
                et = epool.tile([128, 2 * N], F16, name="et")
                if vq == 0:
                    nc.scalar.activation(et[:], sp_cur[:], Exp, bias=BEX[:])
                else:
                    # both heads' [vq:] ranges in one strided-AP
                    # instruction (saves the per-instr ACT overhead)
                    nc.scalar.activation(
                        et[:].rearrange("p (h q) -> p h q", h=2)[:, :, vq:],
                        sp_cur[:].rearrange("p (h q) -> p h q", h=2)[:, :, vq:],
                        Exp,
                        bias=BEX[:],
                    )
                if kc >= 4 * qi:
                    # mask only the 128-wide diagonal sub-block for both
                    # heads in one strided op; the rest of [vq:] is fully
                    # below the diagonal
                    nc.vector.tensor_tensor(
                        et[:].rearrange("p (h q) -> p h q", h=2)[
                            :, :, vq : vq + 128
                        ],
                        et[:].rearrange("p (h q) -> p h q", h=2)[
                            :, :, vq : vq + 128
                        ],
                        MS[:, kc - 4 * qi, None, vq : vq + 128].to_broadcast(
                            (128, 2, 128)
                        ),
                        mybir.AluOpType.mult,
                    )

            if idx + 2 < NIT:
                sp_tiles[idx + 2] = emit_s(*iters[idx + 2])

            if fp8:
                if kc % 2 == 1:
                    need(("v", kc // 4, kc % 4))
                    need(("v", (kc - 1) // 4, (kc - 1) % 4))
                    for hi in range(2):
                        nc.tensor.matmul(
                            avs[hi][:],
                            VA8[:, kc // 2, :, 2 * pr + hi, 0 : D + 1],
                            et2[:, :, hi, :],
                            start=(kc == 1),
                            stop=False,
                            perf_mode=mybir.MatmulPerfMode.DoubleRow,
                        )
            else:
                need(("v", kc // 4, kc % 4))
                for hi in range(2):
                    nc.tensor.matmul(
                        avs[hi][:, vq:],
                        VA[:, kc, 2 * pr + hi, :],
                        et[:, hi * N + vq : (hi + 1) * N],
                        start=(kc == 0),
                        stop=(kc == nkc - 1),
                    )
            drip_advance(sched[idx])

            if kc == nkc - 1:
                last = qi == 3 and pr == 3
                if last:
                    # tail: read PSUM directly (no later user of the banks)
                    # and interleave the two heads' chains to cut latency
                    dns, rbs, rcs = [], [], []
                    for hi in range(2):
                        dn = mpool.tile([1, N], F32, name="dn")
                        nc.vector.tensor_copy(dn[:], avs[hi][D : D + 1, :])
                        dns.append(dn)
                    for hi in range(2):
                        rb = mpool.tile([64, N], F32, name="rb")
                        nc.gpsimd.partition_broadcast(rb[:], dns[hi][:])
                        rbs.append(rb)
                    for hi in range(2):
                        rc = mpool.tile([64, N], F32, name="rc")
                        nc.vector.reciprocal_approx_fast(rc[:], rbs[hi][:])
                        rcs.append(rc)
                    for hi in range(2):
                        off = 64 * hi
                        seg = ON[off : off + 64, pr, qi * N : (qi + 1) * N]
                        nc.vector.tensor_tensor(
                            seg, avs[hi][0:64, :], rcs[hi][:], mybir.AluOpType.mult
                        )
                else:
                    for hi in range(2):
                        off = 64 * hi
                        # one copy releases the accumulator bank; the rest
                        # of the normalize chain runs off SBUF, off the
                        # critical path
                        oc = mpool.tile([D + 1, N], F32, name="oc")
                        nc.vector.tensor_copy(oc[:], avs[hi][:])
                        dn = mpool.tile([1, N], F32, name="dn")
                        nc.vector.tensor_copy(dn[:], oc[D : D + 1, :])
                        rb = mpool.tile([64, N], F32, name="rb")
                        nc.gpsimd.partition_broadcast(rb[:], dn[:])
                        rc = mpool.tile([64, N], F32, name="rc")
                        nc.vector.reciprocal_approx_fast(rc[:], rb[:])
                        seg = ON[off : off + 64, pr, qi * N : (qi + 1) * N]
                        nc.vector.tensor_tensor(
                            seg, oc[0:64, :], rc[:], mybir.AluOpType.mult
                        )
        cur_it[0] = NIT
        while drip:
            drip_advance(1)

        # tail: the last query block's out-projections.  Four blocks' first
        # three contraction chunks (they only need ON pr0-2) issue
        # back-to-back across all four free psum slots, so PE has work
        # queued while the last normalize chain produces ON pr3; their
        # final chunks then land in quick succession.  PSUM->SBUF copies
        # run on the by-now idle Scalar engine.
        def tail_psum(bi):
            if bi % 2 == 1:
                return sps.tile([128, 2 * N], F32, name="sp")[:, 0:N]
            return blkps.tile([128, N], F32, name="blk")

        def tail_finish(ypt, ic, ob):
            nc.tensor.matmul(
                ypt,
                ON[:, 3, ic * 128 : (ic + 1) * 128],
                WO[:, 3, ob * N : (ob + 1) * N],
                start=False,
                stop=True,
            )
            ysb = mpool.tile([128, N], F16, name="ysb")
            nc.scalar.copy(ysb[:], ypt)
            nc.sync.dma_start(
                yp[ic * 128 : (ic + 1) * 128, ob * N : (ob + 1) * N], ysb[:]
            )

        pending = []
        for bi, (ic, ob) in enumerate(tail_blocks):
            if bi in tail_pre:
                pending.append((tail_pre[bi], ic, ob))
            else:
                ypt = tail_psum(bi)
                for cc4 in range(3):
                    nc.tensor.matmul(
                        ypt,
                        ON[:, cc4, ic * 128 : (ic + 1) * 128],
                        WO[:, cc4, ob * N : (ob + 1) * N],
                        start=(cc4 == 0),
                        stop=False,
                    )
                pending.append((ypt, ic, ob))
            if bi >= 3:
                # keep at most 4 prefixes in flight (4 psum slots)
                tail_finish(*pending.pop(0))
        for args in pending:
            tail_finish(*args)

    nc.compile()
    _CACHE["nc"] = nc
    return nc


def _make_masks():
    kp = np.arange(128)[:, None]
    qf = np.arange(N)[None, :]
    m = np.stack([(qf >= kp + 128 * c) for c in range(4)], axis=1)  # [128,4,N]
    return np.ascontiguousarray(m.astype(np.float16))


def _pm(a, chunks):
    """[chunks*128, F] -> partition-major [128, chunks, F] fp16, contiguous."""
    f = a.shape[-1]
    return np.ascontiguousarray(
        a.reshape(chunks, 128, f).transpose(1, 0, 2).astype(np.float16)
    )


def _pm_dc(a):
    """[1024, 512] weight chunk -> [128, 4(dc), 8(cc), 128] fp16."""
    # a[c, j]: c = 8 cc-chunks of 128 partitions; j = 4 dc-chunks of 128
    r = a.reshape(8, 128, 4, 128).transpose(1, 2, 0, 3)  # [128, dc, cc, 128]
    return np.ascontiguousarray(r.astype(np.float16))


def _make_in_maps(x, W_qkv, b_qkv, W_out):
    x = np.asarray(x, dtype=np.float32)
    W_qkv = np.asarray(W_qkv, dtype=np.float32)
    b_qkv = np.asarray(b_qkv, dtype=np.float32)
    W_out = np.asarray(W_out, dtype=np.float32)
    masks = _make_masks()
    xTr = [
        np.ascontiguousarray(
            _pm(x[b].T, 8).reshape(128, 8, 4, N).transpose(0, 2, 1, 3)
        )
        for b in range(B)
    ]
    in_maps = []
    for c in range(NCORES):
        b, g = c // G, c % G
        lo = CPH * g
        bqr = np.ascontiguousarray(
            b_qkv[lo : lo + CPH].reshape(4, 128).T.astype(np.float32)
        )
        bkr = np.ascontiguousarray(
            (SCALE * b_qkv[C + lo : C + lo + CPH]).reshape(4, 128).T.astype(np.float32)
        )
        in_maps.append(
            {
                "xTr": xTr[b],
                "wqr": _pm_dc(W_qkv[:, lo : lo + CPH]),
                "wkr": _pm_dc(W_qkv[:, C + lo : C + lo + CPH]),
                "wvr": _pm(W_qkv[:, 2 * C + lo : 2 * C + lo + CPH], 8),
                "bqr": bqr,
                "bkr": bkr,
                "wor": _pm(W_out[lo : lo + CPH, :], 4),
                "masks": masks,
            }
        )
    return in_maps


def _gather(results, b_out, bias_extra):
    bias = np.asarray(b_out, dtype=np.float32) + bias_extra
    out = np.empty((B, T, C), np.float32)
    for b in range(B):
        out[b] = (
            results[G * b]["yp"].astype(np.float32)
            + results[G * b + 1]["yp"].astype(np.float32)
            + bias[None, :]
        )
    return out


def kernel(x, W_qkv, b_qkv, W_out, b_out, **_):
    nc = _build_program()
    in_maps = _make_in_maps(x, W_qkv, b_qkv, W_out)
    res = bass_utils.run_bass_kernel_spmd(nc, in_maps, core_ids=list(range(NCORES)))
    bias_extra = np.asarray(b_qkv, np.float32)[2 * C :] @ np.asarray(W_out, np.float32)
    return _gather(res.results, b_out, bias_extra)


def kernel_traced(x, W_qkv, b_qkv, W_out, b_out, tmpdir=None, trace=True, **_):
    """Like kernel() but returns (out, exec_time_ns); used by test.py."""
    nc = _build_program()
    in_maps = _make_in_maps(x, W_qkv, b_qkv, W_out)
    res = bass_utils.run_bass_kernel_spmd(
        nc, in_maps, core_ids=list(range(NCORES)), trace=trace, tmpdir=tmpdir
    )
    bias_extra = np.asarray(b_qkv, np.float32)[2 * C :] @ np.asarray(W_out, np.float32)
    return _gather(res.results, b_out, bias_extra), res.exec_time_ns
